# revision 1
# baseline (speedup 1.0000x reference)
"""Multi-head attention (B=4, S=2048, D=1024, H=16) on 8 Trainium2 cores.

Sharding: core = (batch b, head-group g) with 4 batches x 2 groups of 8 heads.
Each core computes, for its batch and its 8 heads:
  QT = (x_q @ Wq_g^T)^T            [512, S]   (feature-major)
  KT = (x_k @ Wk_g^T)^T            [512, S]
  V  =  x_v @ Wv_g^T               [S, 512]   (seq-major, + ones column/head)
  per head h, per q-chunk: scoresT[k, q] = Kh^T.T-contraction, exp on ACT,
  U'T = V'h^T-accum over k (row 64 = softmax denominators),
  attnT = U'T[0:64] * (1/denominator)  (denominator broadcast via K=1 matmul)
  outT_partial = woT.T-contraction over the 512 local features  [D, S]
Host: per batch, sum the two groups' outT partials, transpose, add b_o.

All activations/weights stay fp32 in memory; matmuls run as float32r
(single-pass PE mode, 4x faster than true fp32) by bitcasting the APs.
Softmax skips the max-subtraction (scores are ~N(0,1); exp is safe in fp32
and softmax is shift-invariant).
"""

import ml_dtypes
import numpy as np

import concourse.bass as bass
import concourse.mybir as mybir
import concourse.tile as tile
from concourse import bacc

B = 4
S = 2048
D = 1024
H = 16
DK = 64
NCORES = 8
GROUPS = 2
HPC = H // GROUPS  # heads per core
FC = HPC * DK  # local features per core (512)
P = 128
import os
WEAVE_V = os.environ.get("WEAVE_V", "0") == "1"

F32 = mybir.dt.float32
F32R = mybir.dt.float32r
BF16 = mybir.dt.bfloat16

_NC_CACHE = {}
_RUNNER_CACHE = {}


def build_nc(s=S, d=D, hpc=HPC, bias=False, mm="float32r", nq=512, repeat=1, loop=0):
    """Build the per-core SPMD program. nq = q-chunk width (<=512)."""
    fc = hpc * DK
    mmdt = getattr(mybir.dt, mm)

    ndt = d // P  # d-model tiles (contraction for projections)
    nft = fc // P  # local feature tiles
    nqc = s // nq  # q chunks
    nkt = s // P  # k tiles (seq)
    vw = DK  # per-head V width (denominators via packed ones-matmuls)
    inv_sqrt_dk = 1.0 / float(np.sqrt(DK))

    nc = bacc.Bacc("TRN2", target_bir_lowering=False, debug=False)

    vdt = mmdt if bias else BF16
    xqT = nc.dram_tensor("xqT", [d, s], vdt, kind="ExternalInput").ap()
    xkT = nc.dram_tensor("xkT", [d, s], vdt, kind="ExternalInput").ap()
    xvT = nc.dram_tensor("xvT", [d, s], vdt, kind="ExternalInput").ap()
    wqT = nc.dram_tensor("wqT", [d, fc], vdt, kind="ExternalInput").ap()
    wkT = nc.dram_tensor("wkT", [d, fc], vdt, kind="ExternalInput").ap()
    wvT = nc.dram_tensor("wvT", [d, fc], vdt, kind="ExternalInput").ap()
    woT = nc.dram_tensor("woT", [fc, d], mmdt, kind="ExternalInput").ap()
    outT = nc.dram_tensor("outT", [d, s], F32, kind="ExternalOutput").ap()
    if bias:
        bq = nc.dram_tensor("bq", [1, fc], mmdt, kind="ExternalInput").ap()
        bk = nc.dram_tensor("bk", [1, fc], mmdt, kind="ExternalInput").ap()
        bv = nc.dram_tensor("bv", [1, fc], mmdt, kind="ExternalInput").ap()

    ones_dram = nc.inline_tensor(
        np.ones((1, max(nq, P)), np.float32), name="ones_const"
    ).ap()
    qones_dram = nc.inline_tensor(
        np.ones((P, 4), ml_dtypes.bfloat16), name="qones_const"
    ).ap()

    with tile.TileContext(nc) as tc:
        with (
            tc.tile_pool(name="sb", bufs=1) as sb,
            tc.tile_pool(name="ps", bufs=1, space="PSUM") as ps,
        ):
            import contextlib

            loop_cm = tc.For_i(0, loop, 1) if loop else contextlib.nullcontext()
            with loop_cm:
              for _rep in range(repeat):
                  qt_t = sb.tile([P, nft, s], mmdt, tag="QT")
                  kt_t = sb.tile([P, nft, s], mmdt, tag="KT")
                  vp_t = sb.tile([P, nkt, hpc * vw], BF16, tag="Vp")
                  wo_t = sb.tile([P, fc // P, d], mmdt, tag="wo")
                  ones_t = sb.tile([1, max(nq, P)], mmdt, tag="ones")
                  attnT = qt_t  # attnT(h, qc) overwrites QT columns already consumed

                  def dma_split(dst, src_ap, n):
                      # split a big load into n per-tile DMAs so multiple
                      # DMA engines run in parallel; alternate the issuing
                      # sequencer (sync/gpsimd) so dispatch isn't serialized
                      # on one engine queue
                      for i in range(n):
                          eng = nc.sync if i % 2 == 0 else nc.gpsimd
                          eng.dma_start(out=dst[:, i], in_=src_ap[:, i])

                  nc.sync.dma_start(out=ones_t[:], in_=ones_dram.bitcast(mmdt))
                  ones_bf = sb.tile([P, 4], BF16, tag="onbf")
                  nc.sync.dma_start(out=ones_bf[:], in_=qones_dram[:])
                  if bias:
                      bq_t = sb.tile([1, fc], mmdt, tag="bq")
                      bk_t = sb.tile([1, fc], mmdt, tag="bk")
                      bv_t = sb.tile([1, fc], mmdt, tag="bv")
                      nc.sync.dma_start(out=bq_t[:], in_=bq[:])
                      nc.sync.dma_start(out=bk_t[:], in_=bk[:])
                      nc.sync.dma_start(out=bv_t[:], in_=bv[:])

                  # ---- K projection: KT[f, s_chunk] = sum_d Wk^T[d, f] xk^T[d, s]
                  wk_t = sb.tile([P, ndt, fc], vdt, tag="wproj", bufs=2)
                  dma_split(wk_t, wkT.rearrange("(t p) f -> p t f", p=P), ndt)
                  for sc in range(s // 512):
                      x_t = sb.tile([P, ndt, 512], vdt, tag="xchunk", bufs=int(os.environ.get("XB", "2")))
                      dma_split(
                          x_t,
                          xkT[:, sc * 512 : (sc + 1) * 512].rearrange(
                              "(t p) s -> p t s", p=P
                          ),
                          ndt,
                      )
                      for ft in range(nft):
                          acc = ps.tile([P, 512], F32, tag="sc", bufs=2)
                          if bias:
                              nc.tensor.matmul(
                                  acc[:],
                                  lhsT=bk_t[0:1, ft * P : (ft + 1) * P],
                                  rhs=ones_t[0:1, 0:512],
                                  start=True,
                                  stop=False,
                              )
                          for dt in range(ndt):
                              nc.tensor.matmul(
                                  acc[:],
                                  lhsT=wk_t[:, dt, ft * P : (ft + 1) * P],
                                  rhs=x_t[:, dt, :],
                                  start=(dt == 0 and not bias),
                                  stop=(dt == ndt - 1),
                              )
                          nc.vector.tensor_copy(
                              kt_t[:, ft, sc * 512 : (sc + 1) * 512], acc[:]
                          )

                  # deferred loads (consumers come later than K-proj)
                  wq_t = sb.tile([P, ndt, fc], vdt, tag="wq")
                  dma_split(wq_t, wqT.rearrange("(t p) f -> p t f", p=P), ndt)
                  dma_split(wo_t, woT.rearrange("(t p) j -> p t j", p=P), fc // P)

                  # ---- V projection (seq-major) + ones columns
                  wv_t = sb.tile([P, ndt, fc], vdt, tag="wproj", bufs=2)
                  dma_split(wv_t, wvT.rearrange("(t p) f -> p t f", p=P), ndt)
                  def emit_vproj_tile(st):
                      xv_t = sb.tile([P, ndt, P], vdt, tag="xchunk", bufs=int(os.environ.get("XB", "2")))
                      dma_split(
                          xv_t,
                          xvT[:, st * P : (st + 1) * P].rearrange(
                              "(t p) s -> p t s", p=P
                          ),
                          ndt,
                      )
                      acc = ps.tile([P, fc], F32, tag="pb", bufs=2)
                      if bias:
                          nc.tensor.matmul(
                              acc[:],
                              lhsT=ones_t[0:1, 0:P],
                              rhs=bv_t[0:1, :],
                              start=True,
                              stop=False,
                          )
                      for dt in range(ndt):
                          nc.tensor.matmul(
                              acc[:],
                              lhsT=xv_t[:, dt, :],
                              rhs=wv_t[:, dt, :],
                              start=(dt == 0 and not bias),
                              stop=(dt == ndt - 1),
                          )
                      nc.vector.tensor_copy(vp_t[:, st], acc[:])

                  if not WEAVE_V:
                      for st in range(nkt):
                          emit_vproj_tile(st)

                  # ---- per q-chunk: Q-proj(chunk) -> attention -> O-proj(chunk)
                  # so projection PE work overlaps attention ACT work
                  assert nq == 512
                  for qc in range(nqc):
                      qsl = slice(qc * nq, (qc + 1) * nq)
                      # Q projection for this chunk
                      x_t = sb.tile([P, ndt, 512], vdt, tag="xchunk", bufs=int(os.environ.get("XB", "2")))
                      dma_split(
                          x_t, xqT[:, qsl].rearrange("(t p) s -> p t s", p=P), ndt
                      )
                      for ft in range(nft):
                          acc = ps.tile([P, 512], F32, tag="sc", bufs=2)
                          if bias:
                              nc.tensor.matmul(
                                  acc[:],
                                  lhsT=bq_t[0:1, ft * P : (ft + 1) * P],
                                  rhs=ones_t[0:1, 0:512],
                                  start=True,
                                  stop=False,
                              )
                          for dt in range(ndt):
                              nc.tensor.matmul(
                                  acc[:],
                                  lhsT=wq_t[:, dt, ft * P : (ft + 1) * P],
                                  rhs=x_t[:, dt, :],
                                  start=(dt == 0 and not bias),
                                  stop=(dt == ndt - 1),
                              )
                          nc.vector.tensor_copy(qt_t[:, ft, qsl], acc[:])

                      # attention: head quads (4g..4g+3); scores pairs share
                      # PE row groups, attnV pairs share column groups (M=64
                      # at (0,0)/(0,64)), denominators via 4-way col-packed
                      # M=1 ones-matmuls accumulating in one PSUM bank
                      for g in range(hpc // 4):
                          up0 = ps.tile([P, nq], F32, tag="u", bufs=2, name="up0")
                          up1 = ps.tile([P, nq], F32, tag="u", bufs=2, name="up1")
                          dsm = ps.tile([P, nq], F32, tag="pb", bufs=2, name="dsm")
                          pend = []

                          def flush(kt, et0, et1, g=g, up0=up0, up1=up1, dsm=dsm):
                              for pi, (upx, etx) in enumerate(
                                  ((up0, et0), (up1, et1))
                              ):
                                  hA = 4 * g + 2 * pi
                                  nc.tensor.matmul(
                                      upx[0:64, :],
                                      lhsT=vp_t[:, kt, hA * DK : (hA + 1) * DK],
                                      rhs=etx[:, 0:nq],
                                      tile_position=(0, 0),
                                      start=(kt == 0),
                                      stop=(kt == nkt - 1),
                                  )
                                  nc.tensor.matmul(
                                      upx[64:P, :],
                                      lhsT=vp_t[
                                          :, kt, (hA + 1) * DK : (hA + 2) * DK
                                      ],
                                      rhs=etx[:, nq : 2 * nq],
                                      tile_position=(0, 64),
                                      start=(kt == 0),
                                      stop=(kt == nkt - 1),
                                      skip_group_check=True,
                                  )
                              for j in range(4):
                                  etx = et0 if j < 2 else et1
                                  half = (
                                      slice(0, nq)
                                      if j % 2 == 0
                                      else slice(nq, 2 * nq)
                                  )
                                  nc.tensor.matmul(
                                      dsm[32 * j : 32 * j + 1, :],
                                      lhsT=ones_bf[:, j : j + 1],
                                      rhs=etx[:, half],
                                      tile_position=(0, 32 * j),
                                      start=(kt == 0),
                                      stop=(kt == nkt - 1),
                                      skip_group_check=(j > 0),
                                  )

                          for kt in range(nkt):
                              ets = []
                              for pi in range(2):
                                  tp = 2 * g + pi
                                  pp = ps.tile(
                                      [P, 2 * nq], F32, tag="sc", bufs=2, name="pp"
                                  )
                                  nc.tensor.matmul(
                                      pp[:, 0:nq],
                                      lhsT=kt_t[0:64, tp, kt * P : (kt + 1) * P],
                                      rhs=qt_t[0:64, tp, qsl],
                                      start=True,
                                      stop=True,
                                  )
                                  nc.tensor.matmul(
                                      pp[:, nq : 2 * nq],
                                      lhsT=kt_t[64:P, tp, kt * P : (kt + 1) * P],
                                      rhs=qt_t[64:P, tp, qsl],
                                      start=True,
                                      stop=True,
                                  )
                                  et = sb.tile(
                                      [P, 2 * nq], BF16, tag="exp", bufs=6, name="et"
                                  )
                                  nc.scalar.activation(
                                      et[:],
                                      pp[:],
                                      mybir.ActivationFunctionType.Exp,
                                      scale=inv_sqrt_dk,
                                  )
                                  ets.append(et)
                              pend.append((kt, ets[0], ets[1]))
                              if len(pend) > int(os.environ.get("PEND", "2")):
                                  flush(*pend.pop(0))
                          for e in pend:
                              flush(*e)

                          for j in range(4):
                              h = 4 * g + j
                              tp = h // 2
                              hp = (h % 2) * 64
                              upx = up0 if j < 2 else up1
                              rows = slice(0, 64) if j % 2 == 0 else slice(64, P)
                              rc = sb.tile(
                                  [1, nq], mmdt, tag="recip", bufs=2, name="rc"
                              )
                              with nc.allow_low_precision(
                                  reason="fp32r denominator reciprocal"
                              ):
                                  nc.vector.reciprocal(
                                      rc[:], dsm[32 * j : 32 * j + 1, :]
                                  )
                              pbx = ps.tile(
                                  [64, nq], F32, tag="pb", bufs=2, name="pbx"
                              )
                              nc.tensor.matmul(
                                  pbx[:],
                                  lhsT=ones_t[0:1, 0:64],
                                  rhs=rc[:],
                                  start=True,
                                  stop=True,
                              )
                              bcx = sb.tile(
                                  [64, nq], F32, tag="bcast", bufs=2, name="bcx"
                              )
                              nc.vector.tensor_copy(bcx[:], pbx[:])
                              nc.vector.tensor_mul(
                                  attnT[hp : hp + 64, tp, qsl], upx[rows, :], bcx[:]
                              )

                      # O projection for this q-chunk
                      for jt in range(d // P):
                          acc = ps.tile([P, 512], F32, tag="sc", bufs=2)
                          for ct in range(fc // P):
                              nc.tensor.matmul(
                                  acc[:],
                                  lhsT=wo_t[:, ct, jt * P : (jt + 1) * P],
                                  rhs=attnT[:, ct, qsl],
                                  start=(ct == 0),
                                  stop=(ct == fc // P - 1),
                              )
                          ot = sb.tile([P, 512], F32, tag="out", bufs=2)
                          nc.vector.tensor_copy(ot[:], acc[:])
                          nc.gpsimd.dma_start(
                              out=outT[jt * P : (jt + 1) * P, qsl], in_=ot[:]
                          )

    nc.compile()
    return nc


def _get_nc(bias, mm="float32r"):
    key = (bias, mm)
    if key not in _NC_CACHE:
        _NC_CACHE[key] = build_nc(bias=bias, mm=mm)
    return _NC_CACHE[key]


def make_in_maps(query, key_, value, w_q, b_q, w_k, b_k, w_v, b_v, w_o, b_o):
    bias = bool(np.any(b_q) or np.any(b_k) or np.any(b_v))
    xT = {}
    for b in range(B):
        pdt = np.float32 if bias else ml_dtypes.bfloat16
        xT[("q", b)] = np.ascontiguousarray(query[b].T).astype(pdt)
        xT[("k", b)] = np.ascontiguousarray(key_[b].T).astype(pdt)
        vdt = np.float32 if bias else ml_dtypes.bfloat16
        xT[("v", b)] = np.ascontiguousarray(value[b].T).astype(vdt)
    wT = {}
    for g in range(GROUPS):
        rows = slice(g * FC, (g + 1) * FC)
        pdt = np.float32 if bias else ml_dtypes.bfloat16
        wT[("q", g)] = np.ascontiguousarray(w_q[rows, :].T).astype(pdt)
        wT[("k", g)] = np.ascontiguousarray(w_k[rows, :].T).astype(pdt)
        wT[("v", g)] = np.ascontiguousarray(w_v[rows, :].T).astype(
            np.float32 if bias else ml_dtypes.bfloat16
        )
        wT[("o", g)] = np.ascontiguousarray(w_o[:, rows].T)
    in_maps = []
    for core in range(NCORES):
        b, g = core // GROUPS, core % GROUPS
        m = {
            "xqT": xT[("q", b)],
            "xkT": xT[("k", b)],
            "xvT": xT[("v", b)],
            "wqT": wT[("q", g)],
            "wkT": wT[("k", g)],
            "wvT": wT[("v", g)],
            "woT": wT[("o", g)],
        }
        if bias:
            rows = slice(g * FC, (g + 1) * FC)
            m["bq"] = np.ascontiguousarray(b_q[rows]).reshape(1, FC)
            m["bk"] = np.ascontiguousarray(b_k[rows]).reshape(1, FC)
            m["bv"] = np.ascontiguousarray(b_v[rows]).reshape(1, FC)
        in_maps.append(m)
    return in_maps, bias


def assemble(results, b_o):
    out = np.empty((B, S, D), np.float32)
    for b in range(B):
        acc = results[b * GROUPS]["outT"].copy()
        for g in range(1, GROUPS):
            acc += results[b * GROUPS + g]["outT"]
        out[b] = acc.T
    out += np.asarray(b_o, np.float32)
    return out


def kernel(
    query,
    key_,
    value,
    w_q,
    b_q,
    w_k,
    b_k,
    w_v,
    b_v,
    w_o,
    b_o,
):
    args = [
        np.asarray(a, np.float32)
        for a in (query, key_, value, w_q, b_q, w_k, b_k, w_v, b_v, w_o, b_o)
    ]
    query, key_, value, w_q, b_q, w_k, b_k, w_v, b_v, w_o, b_o = args
    in_maps, bias = make_in_maps(
        query, key_, value, w_q, b_q, w_k, b_k, w_v, b_v, w_o, b_o
    )
    nc = _get_nc(bias)
    from concourse.bass_utils import run_bass_kernel_spmd

    res = run_bass_kernel_spmd(nc, in_maps, list(range(NCORES)))
    return assemble(res.results, b_o)



# revision 2
# speedup vs baseline: 1.2789x; 1.2789x over previous
"""Multi-head attention (B=4, S=2048, D=1024, H=16) on 8 Trainium2 cores.

Sharding: core = (batch b, head-group g) with 4 batches x 2 groups of 8 heads.
Each core computes, for its batch and its 8 heads:
  QT = (x_q @ Wq_g^T)^T            [512, S]   (feature-major, bf16)
  KT = (x_k @ Wk_g^T)^T            [512, S]   (bf16)
  V  =  x_v @ Wv_g^T               [S, 512]   (seq-major, bf16)
  per head h, per q-chunk: scoresT[k, q] via row-group-paired K=64 matmuls,
  exp on ACT (bf16 out), U'T accumulated over k via col-group-paired M=64
  matmuls, denominators via 4-way col-packed M=1 ones-matmuls,
  attnT = U'T * (1/denominator)  (one [128,512] DVE reciprocal, 4-way
  concurrent K=1 broadcast matmuls, two [128,512] DVE muls, bf16 out)
  outT_partial = woT.T-contraction over the 512 local features  [D, S]
Host: per batch, sum the two groups' outT partials, transpose, add b_o.

All PE streams are bf16 (fp32 PSUM accumulate); projections weave into the
first q-chunk's attention so the ACT exp pipeline starts early; O-proj of
chunk qc is deferred into chunk qc+1 so the softmax-normalize tail hides
behind PE work.  Softmax skips max-subtraction (scores ~N(0,1)).
"""

import os

import ml_dtypes
import numpy as np

import concourse.bass as bass
import concourse.mybir as mybir
import concourse.tile as tile
from concourse import bacc

B = 4
S = 2048
D = 1024
H = 16
DK = 64
NCORES = 8
GROUPS = 2
HPC = H // GROUPS  # heads per core
FC = HPC * DK  # local features per core (512)
P = 128

F32 = mybir.dt.float32
F32R = mybir.dt.float32r
BF16 = mybir.dt.bfloat16

_NC_CACHE = {}

PEND = int(os.environ.get("PEND", "2"))
XB = int(os.environ.get("XB", "4"))


def build_nc(s=S, d=D, hpc=HPC, bias=False):
    fc = hpc * DK
    ndt = d // P  # d-model tiles (contraction for projections)
    nft = fc // P  # local feature tiles
    nq = 512
    nqc = s // nq  # q chunks
    nkt = s // P  # k tiles (seq)
    inv_sqrt_dk = 1.0 / float(np.sqrt(DK))

    nc = bacc.Bacc("TRN2", target_bir_lowering=False, debug=False)

    vdt = F32 if bias else BF16
    xqT = nc.dram_tensor("xqT", [d, s], vdt, kind="ExternalInput").ap()
    xkT = nc.dram_tensor("xkT", [d, s], vdt, kind="ExternalInput").ap()
    xvT = nc.dram_tensor("xvT", [d, s], vdt, kind="ExternalInput").ap()
    wqT = nc.dram_tensor("wqT", [d, fc], vdt, kind="ExternalInput").ap()
    wkT = nc.dram_tensor("wkT", [d, fc], vdt, kind="ExternalInput").ap()
    wvT = nc.dram_tensor("wvT", [d, fc], vdt, kind="ExternalInput").ap()
    woT = nc.dram_tensor("woT", [fc, d], vdt, kind="ExternalInput").ap()
    outT = nc.dram_tensor("outT", [d, s], F32, kind="ExternalOutput").ap()
    if bias:
        bq = nc.dram_tensor("bq", [1, fc], F32, kind="ExternalInput").ap()
        bk = nc.dram_tensor("bk", [1, fc], F32, kind="ExternalInput").ap()
        bv = nc.dram_tensor("bv", [1, fc], F32, kind="ExternalInput").ap()

    ones_dram = nc.inline_tensor(
        np.ones((1, max(nq, P)), np.float32), name="ones_const"
    ).ap()
    qones_dram = nc.inline_tensor(
        np.ones((P, 4), ml_dtypes.bfloat16), name="qones_const"
    ).ap()
    ones128_dram = nc.inline_tensor(
        np.ones((P, DK), ml_dtypes.bfloat16), name="ones128_const"
    ).ap()

    with tile.TileContext(nc) as tc:
        with (
            tc.tile_pool(name="sb", bufs=1) as sb,
            tc.tile_pool(name="ps", bufs=1, space="PSUM") as ps,
        ):
            qt_t = sb.tile([P, nft, s], BF16, tag="QT")
            kt_t = sb.tile([P, nft, s], BF16, tag="KT")
            vp_t = sb.tile([P, nkt, fc], BF16, tag="Vp")
            wo_t = sb.tile([P, fc // P, d], vdt, tag="wo")
            ones_t = sb.tile([1, max(nq, P)], F32, tag="ones")
            ones128 = sb.tile([P, DK], BF16, tag="on128")
            attnT = qt_t  # attnT(h, qc) overwrites QT columns already consumed

            def dma_split(dst, src_ap, n):
                # split a big load into n per-tile DMAs so multiple DMA
                # engines run in parallel; alternate the issuing sequencer
                for i in range(n):
                    eng = nc.sync if i % 2 == 0 else nc.gpsimd
                    eng.dma_start(out=dst[:, i], in_=src_ap[:, i])

            nc.sync.dma_start(out=ones_t[:], in_=ones_dram)
            ones_bf = sb.tile([P, 4], BF16, tag="onbf")
            nc.sync.dma_start(out=ones_bf[:], in_=qones_dram[:])
            nc.sync.dma_start(out=ones128[:], in_=ones128_dram[:])
            if bias:
                bq_t = sb.tile([1, fc], F32, tag="bq")
                bk_t = sb.tile([1, fc], F32, tag="bk")
                bv_t = sb.tile([1, fc], F32, tag="bv")
                nc.sync.dma_start(out=bq_t[:], in_=bq[:])
                nc.sync.dma_start(out=bk_t[:], in_=bk[:])
                nc.sync.dma_start(out=bv_t[:], in_=bv[:])

            # ---- weight loads
            wk_t = sb.tile([P, ndt, fc], vdt, tag="wk")
            dma_split(wk_t, wkT.rearrange("(t p) f -> p t f", p=P), ndt)
            wq_t = sb.tile([P, ndt, fc], vdt, tag="wq")
            dma_split(wq_t, wqT.rearrange("(t p) f -> p t f", p=P), ndt)
            wv_t = sb.tile([P, ndt, fc], vdt, tag="wv")
            dma_split(wv_t, wvT.rearrange("(t p) f -> p t f", p=P), ndt)
            dma_split(wo_t, woT.rearrange("(t p) j -> p t j", p=P), fc // P)

            # ---- K projection chunk: KT[f, sc*512:(sc+1)*512]
            def kproj_chunk(sc):
                x_t = sb.tile([P, ndt, 512], vdt, tag="xchunk", bufs=XB)
                dma_split(
                    x_t,
                    xkT[:, sc * 512 : (sc + 1) * 512].rearrange(
                        "(t p) s -> p t s", p=P
                    ),
                    ndt,
                )
                for ft in range(nft):
                    acc = ps.tile([P, 2, nq], F32, tag="sc", bufs=2)
                    if bias:
                        nc.tensor.matmul(
                            acc[:, 0],
                            lhsT=bk_t[0:1, ft * P : (ft + 1) * P],
                            rhs=ones_t[0:1, 0:512],
                            start=True,
                            stop=False,
                        )
                    for dt in range(ndt):
                        nc.tensor.matmul(
                            acc[:, 0],
                            lhsT=wk_t[:, dt, ft * P : (ft + 1) * P],
                            rhs=x_t[:, dt, :],
                            start=(dt == 0 and not bias),
                            stop=(dt == ndt - 1),
                        )
                    nc.vector.tensor_copy(
                        kt_t[:, ft, sc * 512 : (sc + 1) * 512], acc[:, 0]
                    )

            # ---- Q projection for one q-chunk (one ft tile at a time)
            def qproj_ft(qc, ft, x_t):
                qsl = slice(qc * nq, (qc + 1) * nq)
                acc = ps.tile([P, 2, nq], F32, tag="sc", bufs=2)
                if bias:
                    nc.tensor.matmul(
                        acc[:, 0],
                        lhsT=bq_t[0:1, ft * P : (ft + 1) * P],
                        rhs=ones_t[0:1, 0:512],
                        start=True,
                        stop=False,
                    )
                for dt in range(ndt):
                    nc.tensor.matmul(
                        acc[:, 0],
                        lhsT=wq_t[:, dt, ft * P : (ft + 1) * P],
                        rhs=x_t[:, dt, :],
                        start=(dt == 0 and not bias),
                        stop=(dt == ndt - 1),
                    )
                nc.vector.tensor_copy(qt_t[:, ft, qsl], acc[:, 0])

            def qproj_x(qc):
                qsl = slice(qc * nq, (qc + 1) * nq)
                x_t = sb.tile([P, ndt, 512], vdt, tag="xchunk", bufs=XB)
                dma_split(
                    x_t, xqT[:, qsl].rearrange("(t p) s -> p t s", p=P), ndt
                )
                return x_t

            # ---- V projection (seq-major): one k-tile of 128 seq positions
            def vproj_tile(st):
                xv_t = sb.tile([P, ndt, P], vdt, tag="xvchunk", bufs=XB)
                dma_split(
                    xv_t,
                    xvT[:, st * P : (st + 1) * P].rearrange(
                        "(t p) s -> p t s", p=P
                    ),
                    ndt,
                )
                acc = ps.tile([P, 2, nq], F32, tag="sc", bufs=2)
                if bias:
                    nc.tensor.matmul(
                        acc[:, 0],
                        lhsT=ones_t[0:1, 0:P],
                        rhs=bv_t[0:1, :],
                        start=True,
                        stop=False,
                    )
                for dt in range(ndt):
                    nc.tensor.matmul(
                        acc[:, 0],
                        lhsT=xv_t[:, dt, :],
                        rhs=wv_t[:, dt, :],
                        start=(dt == 0 and not bias),
                        stop=(dt == ndt - 1),
                    )
                nc.vector.tensor_copy(vp_t[:, st], acc[:, 0])

            # ---- O projection for one output jt tile of a finished q-chunk
            def oproj_jt(qc, jt):
                qsl = slice(qc * nq, (qc + 1) * nq)
                acc = ps.tile([P, 2, nq], F32, tag="sc", bufs=2)
                for ct in range(fc // P):
                    nc.tensor.matmul(
                        acc[:, 0],
                        lhsT=wo_t[:, ct, jt * P : (jt + 1) * P],
                        rhs=attnT[:, ct, qsl],
                        start=(ct == 0),
                        stop=(ct == fc // P - 1),
                    )
                ot = sb.tile([P, 512], F32, tag="out", bufs=2)
                nc.vector.tensor_copy(ot[:], acc[:, 0])
                nc.gpsimd.dma_start(out=outT[jt * P : (jt + 1) * P, qsl], in_=ot[:])

            # ---- attention for one head-quad g (heads 4g..4g+3) over one
            # q-chunk; `weave` is called between kt steps to interleave
            # projection work on the PE
            def attn_quad(qc, g, weave):
                qsl = slice(qc * nq, (qc + 1) * nq)
                up0 = ps.tile([P, nq], F32, tag="u", bufs=2, name="up0")
                up1 = ps.tile([P, nq], F32, tag="u", bufs=2, name="up1")
                dsm = ps.tile([P, nq], F32, tag="d", bufs=2, name="dsm")
                pend = []

                def flush(kt, et0, et1):
                    for pi, (upx, etx) in enumerate(((up0, et0), (up1, et1))):
                        hA = 4 * g + 2 * pi
                        nc.tensor.matmul(
                            upx[0:64, :],
                            lhsT=vp_t[:, kt, hA * DK : (hA + 1) * DK],
                            rhs=etx[:, 0:nq],
                            tile_position=(0, 0),
                            start=(kt == 0),
                            stop=(kt == nkt - 1),
                        )
                        nc.tensor.matmul(
                            upx[64:P, :],
                            lhsT=vp_t[:, kt, (hA + 1) * DK : (hA + 2) * DK],
                            rhs=etx[:, nq : 2 * nq],
                            tile_position=(0, 64),
                            start=(kt == 0),
                            stop=(kt == nkt - 1),
                            skip_group_check=True,
                        )
                    for j in range(4):
                        etx = et0 if j < 2 else et1
                        half = slice(0, nq) if j % 2 == 0 else slice(nq, 2 * nq)
                        nc.tensor.matmul(
                            dsm[32 * j : 32 * j + 1, :],
                            lhsT=ones_bf[:, j : j + 1],
                            rhs=etx[:, half],
                            tile_position=(0, 32 * j),
                            start=(kt == 0),
                            stop=(kt == nkt - 1),
                            skip_group_check=(j > 0),
                        )

                for kt in range(nkt):
                    ets = []
                    for pi in range(2):
                        tp = 2 * g + pi
                        pp = ps.tile([P, 2, nq], F32, tag="sc", bufs=2, name="pp")
                        nc.tensor.matmul(
                            pp[:, 0],
                            lhsT=kt_t[0:64, tp, kt * P : (kt + 1) * P],
                            rhs=qt_t[0:64, tp, qsl],
                            start=True,
                            stop=True,
                        )
                        nc.tensor.matmul(
                            pp[:, 1],
                            lhsT=kt_t[64:P, tp, kt * P : (kt + 1) * P],
                            rhs=qt_t[64:P, tp, qsl],
                            start=True,
                            stop=True,
                        )
                        et = sb.tile(
                            [P, 2 * nq], BF16, tag="exp", bufs=6, name="et"
                        )
                        nc.scalar.activation(
                            et[:],
                            pp[:].rearrange("p t n -> p (t n)"),
                            mybir.ActivationFunctionType.Exp,
                            scale=inv_sqrt_dk,
                        )
                        ets.append(et)
                    pend.append((kt, ets[0], ets[1]))
                    if len(pend) > PEND:
                        flush(*pend.pop(0))
                    weave(kt)
                for e in pend:
                    flush(*e)

                # ---- normalize: attnT[h] = up[h] / denom[h]
                rc_sb = sb.tile([P, nq], BF16, tag="recip", bufs=2, name="rc")
                with nc.allow_low_precision(reason="softmax denom reciprocal"):
                    nc.vector.reciprocal(rc_sb[:], dsm[:])
                pb = ps.tile([P, 2, nq], F32, tag="sc", bufs=2, name="pb")
                for j in range(4):
                    nc.tensor.matmul(
                        pb[64 * (j % 2) : 64 * (j % 2) + 64, j // 2],
                        lhsT=ones128[32 * j : 32 * j + 1, :],
                        rhs=rc_sb[32 * j : 32 * j + 1, :],
                        tile_position=(32 * j, 64 * (j % 2)),
                        start=True,
                        stop=True,
                        skip_group_check=(j > 0),
                    )
                bc = sb.tile([P, 2, nq], F32, tag="bcast", bufs=2, name="bc")
                nc.vector.tensor_copy(
                    bc[:].rearrange("p t n -> p (t n)"),
                    pb[:].rearrange("p t n -> p (t n)"),
                )
                for pi, upx in enumerate((up0, up1)):
                    nc.vector.tensor_mul(
                        attnT[:, 2 * g + pi, qsl], upx[:], bc[:, pi]
                    )

            # ================= emission schedule =================
            kproj_chunk(0)
            xq0 = qproj_x(0)
            for ft in range(nft):
                qproj_ft(0, ft, xq0)

            for qc in range(nqc):
                # g=0 quad; weave V-proj (qc 0) / deferred O-proj (qc>0)
                if qc == 0:

                    def weave_g0(kt, qc=qc):
                        vproj_tile(kt)
                        if kt in (1, 3, 5):
                            kproj_chunk((kt + 1) // 2)

                else:

                    def weave_g0(kt, qc=qc):
                        if kt % 2 == 0:
                            oproj_jt(qc - 1, kt // 2)

                attn_quad(qc, 0, weave_g0)

                # g=1 quad; weave next chunk's Q projection
                if qc + 1 < nqc:
                    xq_next = qproj_x(qc + 1)

                    def weave_g1(kt, qc=qc, x=xq_next):
                        if kt in (4, 8, 12, 15):
                            qproj_ft(qc + 1, (kt // 4) - 1 if kt != 15 else 3, x)

                else:

                    def weave_g1(kt):
                        pass

                attn_quad(qc, 1, weave_g1)

            for jt in range(d // P):
                oproj_jt(nqc - 1, jt)

    nc.compile()
    return nc


def _get_nc(bias):
    if bias not in _NC_CACHE:
        _NC_CACHE[bias] = build_nc(bias=bias)
    return _NC_CACHE[bias]


def make_in_maps(query, key_, value, w_q, b_q, w_k, b_k, w_v, b_v, w_o, b_o):
    bias = bool(np.any(b_q) or np.any(b_k) or np.any(b_v))
    pdt = np.float32 if bias else ml_dtypes.bfloat16
    xT = {}
    for b in range(B):
        xT[("q", b)] = np.ascontiguousarray(query[b].T).astype(pdt)
        xT[("k", b)] = np.ascontiguousarray(key_[b].T).astype(pdt)
        xT[("v", b)] = np.ascontiguousarray(value[b].T).astype(pdt)
    wT = {}
    for g in range(GROUPS):
        rows = slice(g * FC, (g + 1) * FC)
        wT[("q", g)] = np.ascontiguousarray(w_q[rows, :].T).astype(pdt)
        wT[("k", g)] = np.ascontiguousarray(w_k[rows, :].T).astype(pdt)
        wT[("v", g)] = np.ascontiguousarray(w_v[rows, :].T).astype(pdt)
        wT[("o", g)] = np.ascontiguousarray(w_o[:, rows].T).astype(pdt)
    in_maps = []
    for core in range(NCORES):
        b, g = core // GROUPS, core % GROUPS
        m = {
            "xqT": xT[("q", b)],
            "xkT": xT[("k", b)],
            "xvT": xT[("v", b)],
            "wqT": wT[("q", g)],
            "wkT": wT[("k", g)],
            "wvT": wT[("v", g)],
            "woT": wT[("o", g)],
        }
        if bias:
            rows = slice(g * FC, (g + 1) * FC)
            m["bq"] = np.ascontiguousarray(b_q[rows]).reshape(1, FC)
            m["bk"] = np.ascontiguousarray(b_k[rows]).reshape(1, FC)
            m["bv"] = np.ascontiguousarray(b_v[rows]).reshape(1, FC)
        in_maps.append(m)
    return in_maps, bias


def assemble(results, b_o):
    out = np.empty((B, S, D), np.float32)
    for b in range(B):
        acc = results[b * GROUPS]["outT"].copy()
        for g in range(1, GROUPS):
            acc += results[b * GROUPS + g]["outT"]
        out[b] = acc.T
    out += np.asarray(b_o, np.float32)
    return out


def kernel(
    query,
    key_,
    value,
    w_q,
    b_q,
    w_k,
    b_k,
    w_v,
    b_v,
    w_o,
    b_o,
):
    args = [
        np.asarray(a, np.float32)
        for a in (query, key_, value, w_q, b_q, w_k, b_k, w_v, b_v, w_o, b_o)
    ]
    query, key_, value, w_q, b_q, w_k, b_k, w_v, b_v, w_o, b_o = args
    in_maps, bias = make_in_maps(
        query, key_, value, w_q, b_q, w_k, b_k, w_v, b_v, w_o, b_o
    )
    nc = _get_nc(bias)
    from concourse.bass_utils import run_bass_kernel_spmd

    res = run_bass_kernel_spmd(nc, in_maps, list(range(NCORES)))
    return assemble(res.results, b_o)
